# revision 1
# baseline (speedup 1.0000x reference)
"""Trainium2 Bass kernel for nn_Decoder_gru_2_8589935086.

Computes, for all M=3486 unordered pairs (i<j) of the N=84 graph nodes:
GRUCell(x[i], x[j]) -> 3x (Linear -> ReLU -> full-tensor LayerNorm) -> Linear
-> sigmoid, scattered into a symmetric [84, 84] matrix.

Strategy (single NeuronCore; the three LayerNorms are over the FULL [M, H]
tensor, so a sharded version needs 3 sequential cross-core all-reduces whose
~7-20us-each latency floor dwarfs this tiny workload):
  * Pair expansion commutes with the GRU input/hidden matmuls: compute
    A = x@W_ih.T, B = x@W_hh.T ([84, 192]) once, then gather rows per-pair
    with one-hot selection-matrix matmuls (fp32r, 1 cycle/row) accumulating
    A[iu] + B[ju] directly in PSUM.  Biases ride along as an extra
    all-ones row in the selection matrices.
  * Everything lives transposed [feature on partitions, pair on free], with
    the M=3486 pairs packed as two halves -> [128, 1743]; MLP layers are
    single matmuls against host-built block-diagonal weights, so no
    activation transposes anywhere.
  * Full-tensor LayerNorm is folded into the next layer:
    ln(y)@W.T = a*(y@W.T) - a*m*rowsum(W), with sum(y) free via the ReLU
    evacuation's accum_out and sum(y^2) via one tensor_tensor_reduce pass.
    rsqrt(var+eps) is computed on the vector engine (reciprocal + seeded
    Newton iterations) to avoid ACT table-set switches.
"""

import sys
import os

for _p in ("/opt/trn_rl_repo",):
    if _p not in sys.path and os.path.isdir(_p):
        sys.path.insert(0, _p)

import numpy as np

N = 84
H = 64
M = N * (N - 1) // 2  # 3486
F = M // 2            # 1743 per half
EPS = 1e-5
CHUNKS = [(0, 448), (448, 448), (896, 448), (1344, 399)]
# Newton rsqrt seed y0 = RA/v + RB + RC*v (16.6% max rel err on [0.04, 6]),
# 4 iterations -> fp32-exact.
RA, RB, RC = 0.19709184, 0.90519586, -0.09958437
NR_ITERS = 3
PKW = 1174

_IU, _JU = np.triu_indices(N, k=1)

_prog_cache = {}


def _build_program(dbg=False):
    INTERLEAVE = os.environ.get("K_IL", "1") == "1"
    import concourse.bacc as bacc
    import concourse.mybir as mybir
    from concourse import tile

    f32 = mybir.dt.float32
    f16 = mybir.dt.float16
    AF = mybir.ActivationFunctionType
    OP = mybir.AluOpType

    nc = bacc.Bacc("TRN2", target_bir_lowering=False, debug=False)

    def din(name, shape, dt=f16):
        return nc.dram_tensor(name, list(shape), dt, kind="ExternalInput")

    pk_d = din("pack16", (128, PKW))
    scmb_d = [din(f"scmb{ci}", (N + 1, 4 * cw)) for ci, (c0, cw) in enumerate(CHUNKS)]
    consts_d = din("consts", (128, 288), f32)
    out_d = nc.dram_tensor("o", [2, F], f32, kind="ExternalOutput")
    dbg_d = {}
    if dbg:
        for nm, shp in [("h", [128, F]), ("y1", [128, F]), ("y2", [H, F]),
                        ("y3", [H, F]), ("rz", [128, F * 2]), ("nn", [128, F]),
                        ("ST1", [128, 8]), ("ST2", [H, 8]), ("ST3", [H, 8]),
                        ("bcd1", [128, 2]), ("bcd2", [128, 2]), ("bcd3", [128, 2])]:
            dbg_d[nm] = nc.dram_tensor("dbg_" + nm, shp, f32, kind="ExternalOutput")

    with tile.TileContext(nc) as tc:
        with (
            tc.tile_pool(name="cons", bufs=1) as cons,
            tc.tile_pool(name="spool", bufs=1) as spool,
            tc.tile_pool(name="big", bufs=1) as big,
            tc.tile_pool(name="scr", bufs=2) as scr,
            tc.tile_pool(name="nrp", bufs=1) as nrp,
            tc.tile_pool(name="psrz", bufs=1, space="PSUM") as psrz,
            tc.tile_pool(name="psnb", bufs=1, space="PSUM") as psnb,
            tc.tile_pool(name="psm", bufs=2, space="PSUM") as psm,
            tc.tile_pool(name="pss", bufs=1, space="PSUM") as pss,
        ):
            # ---- persistent SBUF tiles ----
            # pack16 layout (cols): xT [0:84) r0:64, x [84:148) r0:84,
            # wih [148:340) r0:64, whh [340:532) r0:64, w1bd [532:660) r0:128,
            # w2bd [660:724) r0:128, w3bd [724:788) r0:64, w4bd [788:790) r0:64,
            # biasA [790:982) row84, biasB [982:1174) row84
            pk = cons.tile([128, PKW], f16, tag="pk")
            xT_t = pk[0:H, 0:84]
            x_t = pk[0:N, 84:148]
            wih_t = pk[0:H, 148:340]
            whh_t = pk[0:H, 340:532]
            w1bd = pk[:, 532:660]
            w2bd = pk[:, 660:724]
            w3bd = pk[0:H, 724:788]
            w4bd = pk[0:H, 788:790]
            LA = cons.tile([N + 1, 3 * H], f16, tag="LA")
            LB = cons.tile([N + 1, 3 * H], f16, tag="LB")
            consts = cons.tile([128, 288], f32, tag="consts")

            scmb_t = []
            siu_t = {}
            sju_t = {}
            for ci, (c0, cw) in enumerate(CHUNKS):
                st = spool.tile([N + 1, 4 * cw], f16, tag=f"scmb{ci}", name=f"scmb{ci}")
                scmb_t.append(st)
                # layout: [siu_T | sju_T | siu_B | sju_B]
                siu_t[ci, 0] = st[:, 0:cw]
                sju_t[ci, 0] = st[:, cw:2 * cw]
                siu_t[ci, 1] = st[:, 2 * cw:3 * cw]
                sju_t[ci, 1] = st[:, 3 * cw:4 * cw]

            y1T = big.tile([128, F], f16, tag="y1T")
            y2T = big.tile([H, F], f16, tag="y2T")
            y3T = big.tile([H, F], f16, tag="y3T")
            oT = big.tile([2, F], f32, tag="oT")
            ST1 = big.tile([128, 8], f32, tag="ST1")
            ST2 = big.tile([H, 8], f32, tag="ST2")
            ST3 = big.tile([H, 8], f32, tag="ST3")

            ones_col = consts[:, 0:1]
            b1col = consts[:, 1:2]
            zcol = consts[:, 8:9]
            onecell = consts[0:1, 0:1]
            # c-col matmul lhsT rows (partition 0, fp32)
            w2row = consts[0:1, 16:80]
            b2row = consts[0:1, 80:144]
            w3row = consts[0:1, 144:208]
            b3row = consts[0:1, 208:272]
            w4row = consts[0:1, 272:274]
            b4row = consts[0:1, 274:276]
            ones2row = consts[0:1, 276:278]

            # ---- input DMAs: critical-first, spread across 3 DGE queues ----
            # table preload: dummy sigmoid on a memset cell (no DMA dep)
            wsrc = nrp.tile([1, 1], f32, tag="wsrc")
            nc.vector.memset(wsrc[:], 0.0)
            warm = nrp.tile([1, 1], f32, tag="warm")
            nc.scalar.activation(warm[:], wsrc[:], AF.Sigmoid)
            PS = 43  # scmb partition split point
            nc.sync.dma_start(LA[N:N + 1, :], pk_d.ap()[N:N + 1, 790:982])
            nc.sync.dma_start(LB[N:N + 1, :], pk_d.ap()[N:N + 1, 982:1174])
            nc.gpsimd.dma_start(scmb_t[0][0:PS, :], scmb_d[0].ap()[0:PS, :])
            nc.scalar.dma_start(scmb_t[0][PS:N + 1, :], scmb_d[0].ap()[PS:N + 1, :])
            nc.sync.dma_start(pk[:, 0:532], pk_d.ap()[:, 0:532])
            nc.gpsimd.dma_start(pk[:, 532:790], pk_d.ap()[:, 532:790])
            nc.scalar.dma_start(consts[:], consts_d.ap())
            for ci in range(1, len(CHUNKS)):
                eng = (nc.sync, nc.gpsimd, nc.scalar)[ci % 3]
                eng2 = (nc.gpsimd, nc.scalar, nc.sync)[ci % 3]
                eng.dma_start(scmb_t[ci][0:PS, :], scmb_d[ci].ap()[0:PS, :])
                eng2.dma_start(scmb_t[ci][PS:N + 1, :], scmb_d[ci].ap()[PS:N + 1, :])

            # ---- A0 = x@W_ih.T, B0 = x@W_hh.T  (into LA/LB rows 0:84) ----
            pA0 = psnb.tile([N, 3 * H], f32, tag="p_An", padded_shape=[N, 512])
            nc.tensor.matmul(pA0[:], xT_t[:], wih_t[:], start=True, stop=True)
            nc.vector.tensor_scalar(LA[0:N, :], pA0[:], 1.0, None, OP.mult)
            pB0 = psnb.tile([N, 3 * H], f32, tag="p_Bn", padded_shape=[N, 512])
            nc.tensor.matmul(pB0[:], xT_t[:], whh_t[:], start=True, stop=True)
            nc.vector.tensor_scalar(LB[0:N, :], pB0[:], 1.0, None, OP.mult)

            # ---- GRU + L1, chunk by chunk (emission software-pipelined) ----
            def gru_chunk_mm(ci):
                c0, cw = CHUNKS[ci]
                csl = slice(c0, c0 + cw)
                # r gate in bank 0 ([0:cw]), z gate in bank 1 ([512:512+cw])
                p_rz = psrz.tile([128, 1024], f32, tag="p_rz")
                p_An = psnb.tile([128, cw], f32, tag="p_An", padded_shape=[128, 512])
                p_Bn = psnb.tile([128, cw], f32, tag="p_Bn", padded_shape=[128, 512])
                p_x2 = psnb.tile([128, cw], f32, tag="p_x2", padded_shape=[128, 512])

                if INTERLEAVE:
                    PO = (slice(0, 64), slice(64, 128))
                    TP = ((0, 0), (0, 64))
                    for L, gsl, dst, ss in (
                        (LA, slice(0, 64), lambda hi: p_rz[PO[hi], 0:cw], siu_t),
                        (LB, slice(0, 64), lambda hi: p_rz[PO[hi], 0:cw], sju_t),
                        (LA, slice(64, 128), lambda hi: p_rz[PO[hi], 512:512 + cw], siu_t),
                        (LB, slice(64, 128), lambda hi: p_rz[PO[hi], 512:512 + cw], sju_t),
                        (LA, slice(128, 192), lambda hi: p_An[PO[hi], :], siu_t),
                        (LB, slice(128, 192), lambda hi: p_Bn[PO[hi], :], sju_t),
                    ):
                        for hi in range(2):
                            if gsl == slice(128, 192):
                                s_, p_ = True, True
                            else:
                                s_, p_ = (True, False) if L is LA else (False, True)
                            nc.tensor.matmul(dst(hi), L[:, gsl], ss[ci, hi][:],
                                             start=s_, stop=p_, tile_position=TP[hi],
                                             skip_group_check=True)
                else:
                    for hi in range(2):
                        po = slice(64 * hi, 64 * hi + 64)
                        tp = (0, 64 * hi)
                        siu = siu_t[ci, hi]
                        sju = sju_t[ci, hi]
                        nc.tensor.matmul(p_rz[po, 0:cw], LA[:, 0:64], siu[:],
                                         start=True, stop=False, tile_position=tp)
                        nc.tensor.matmul(p_rz[po, 0:cw], LB[:, 0:64], sju[:],
                                         start=False, stop=True, tile_position=tp)
                        nc.tensor.matmul(p_rz[po, 512:512 + cw], LA[:, 64:128], siu[:],
                                         start=True, stop=False, tile_position=tp)
                        nc.tensor.matmul(p_rz[po, 512:512 + cw], LB[:, 64:128], sju[:],
                                         start=False, stop=True, tile_position=tp)
                        nc.tensor.matmul(p_An[po, :], LA[:, 128:192], siu[:],
                                         start=True, stop=True, tile_position=tp)
                        nc.tensor.matmul(p_Bn[po, :], LB[:, 128:192], sju[:],
                                         start=True, stop=True, tile_position=tp)

                for hi in range(2):
                    nc.tensor.matmul(p_x2[PO[hi], :], x_t[:], sju_t[ci, hi][0:N, :],
                                     start=True, stop=True, tile_position=TP[hi],
                                     skip_group_check=True)
                return p_rz, p_An, p_Bn, p_x2

            def gru_chunk_ew(ci, p_rz, p_An, p_Bn, p_x2):
                c0, cw = CHUNKS[ci]
                csl = slice(c0, c0 + cw)
                rz_c = scr.tile([128, 2 * cw], f16, tag="rz", name="rz")
                s_c = scr.tile([128, cw], f16, tag="s")
                s2_c = scr.tile([128, cw], f16, tag="s2")
                nn_c = scr.tile([128, cw], f16, tag="nn")
                zx2_c = scr.tile([128, cw], f16, tag="zx2")
                q_c = scr.tile([128, cw], f16, tag="q")
                h_c = scr.tile([128, cw], f16, tag="h")
                dump_c = scr.tile([128, cw], f16, tag="dump")

                rz_src = p_rz[:].rearrange("p (b k) -> p b k", b=2)[:, :, 0:cw]
                rz_dst = rz_c[:].rearrange("p (b k) -> p b k", b=2)
                r_sl = rz_c[:, 0:cw]
                z_sl = rz_c[:, cw:2 * cw]

                nc.scalar.activation(rz_dst, rz_src, AF.Sigmoid)
                nc.vector.tensor_tensor(s_c[:], r_sl, p_Bn[:], OP.mult)
                nc.vector.tensor_tensor(s2_c[:], s_c[:], p_An[:], OP.add)
                nc.scalar.activation(nn_c[:], s2_c[:], AF.Tanh)
                # zx2 = z*x2 ; q = (z-1)*nn ; h = zx2 - q   (gpsimd, fp16 sbuf)
                nc.vector.tensor_tensor(zx2_c[:], z_sl, p_x2[:], OP.mult)
                nc.vector.scalar_tensor_tensor(q_c[:], z_sl, 1.0, nn_c[:],
                                               OP.subtract, OP.mult)
                nc.gpsimd.tensor_tensor(h_c[:], zx2_c[:], q_c[:], OP.subtract)

                p_l1 = psm.tile([128, cw], f32, tag="p_l", padded_shape=[128, 512])
                if dbg:
                    dbg_h = scr.tile([128, cw], f32, tag="dbg", name="dbg_h")
                    nc.vector.tensor_scalar(dbg_h[:], h_c[:], 1.0, None, OP.mult)
                    nc.sync.dma_start(dbg_d["h"].ap()[:, csl], dbg_h[:])
                    dbg_rz = scr.tile([128, 2 * cw], f32, tag="dbgrz", name="dbg_rz")
                    nc.vector.tensor_scalar(dbg_rz[:], rz_c[:], 1.0, None, OP.mult)
                    nc.sync.dma_start(dbg_d["rz"].ap()[:, 2 * c0:2 * c0 + 2 * cw], dbg_rz[:])
                    dbg_nn = scr.tile([128, cw], f32, tag="dbg", name="dbg_nn")
                    nc.vector.tensor_scalar(dbg_nn[:], nn_c[:], 1.0, None, OP.mult)
                    nc.sync.dma_start(dbg_d["nn"].ap()[:, csl], dbg_nn[:])
                nc.tensor.matmul(p_l1[:], w1bd[:], h_c[:], start=True, stop=True)
                # y1 = relu(p + b1)  on DVE, sum via accum
                nc.vector.scalar_tensor_tensor(y1T[:, csl], p_l1[:], b1col,
                                               zcol.broadcast_to((128, cw)),
                                               OP.add, OP.max,
                                               accum_out=ST1[:, ci:ci + 1])
                nc.scalar.activation(dump_c[:], y1T[:, csl], AF.Square,
                                     accum_out=ST1[:, 4 + ci:5 + ci])

            pending = None
            for ci in range(len(CHUNKS)):
                ps = gru_chunk_mm(ci)
                if pending is not None:
                    gru_chunk_ew(pending[0], *pending[1])
                pending = (ci, ps)
            gru_chunk_ew(pending[0], *pending[1])

            # ---- LayerNorm scalar chains (scale-migrated) ----
            def ln_chain(ST, parts, cnt, idx, Gprev=None, Gprev_sq=None):
                """Returns (mq, G, Gsq, sinv): hat-mean/q in mq, cumulative
                rsqrt product G = a1..ak, its square, and 1/G."""
                p_s = pss.tile([1, 8], f32, tag="p_s", padded_shape=[1, 512],
                               name=f"p_s{idx}")
                nc.tensor.matmul(p_s[:], ones_col[0:parts, :], ST[:],
                                 start=True, stop=True)
                sums = nrp.tile([1, 2], f32, tag=f"sums{idx}", name=f"sums{idx}")
                nc.vector.tensor_reduce(
                    sums[:], p_s[:].rearrange("p (a b) -> p a b", a=2),
                    axis=mybir.AxisListType.X, op=OP.add)
                mq = nrp.tile([1, 2], f32, tag=f"mq{idx}", name=f"mq{idx}")
                nc.vector.tensor_scalar(mq[:], sums[:], 1.0 / cnt, None, OP.mult)
                m2 = nrp.tile([1, 1], f32, tag=f"m2{idx}", name=f"m2{idx}")
                nc.vector.tensor_scalar(m2[:], mq[:, 0:1], mq[:, 0:1], None, OP.mult)
                d_t = nrp.tile([1, 1], f32, tag=f"d{idx}", name=f"d{idx}")
                nc.vector.scalar_tensor_tensor(d_t[:], m2[:], -1.0, mq[:, 1:2],
                                               OP.mult, OP.add)
                v_t = nrp.tile([1, 1], f32, tag=f"v{idx}", name=f"v{idx}")
                nc.vector.tensor_scalar(v_t[:], d_t[:],
                                        Gprev_sq[:] if Gprev_sq is not None else 1.0,
                                        EPS, OP.mult, OP.add)
                # off-critical-path helpers first so they overlap the chain
                vqs = []
                for k in range(NR_ITERS):
                    vq = nrp.tile([1, 1], f32, tag=f"vq{idx}_{k}", name=f"vq{idx}_{k}")
                    nc.vector.tensor_scalar(vq[:], v_t[:], 0.25 ** k, None, OP.mult)
                    vqs.append(vq)
                rv = nrp.tile([1, 1], f32, tag=f"rv{idx}", name=f"rv{idx}")
                nc.vector.reciprocal(rv[:], v_t[:])
                t1 = nrp.tile([1, 1], f32, tag=f"t1{idx}", name=f"t1{idx}")
                nc.vector.tensor_scalar(t1[:], v_t[:], RC, RB, OP.mult, OP.add)
                w_t = nrp.tile([1, 1], f32, tag=f"w{idx}", name=f"w{idx}")
                nc.vector.scalar_tensor_tensor(w_t[:], rv[:], RA, t1[:],
                                               OP.mult, OP.add)
                t_t = nrp.tile([1, 1], f32, tag=f"t{idx}", name=f"t{idx}")
                for k in range(NR_ITERS):
                    nc.vector.tensor_scalar(t_t[:], w_t[:], w_t[:], vqs[k][:],
                                            OP.mult, OP.mult)
                    nc.vector.scalar_tensor_tensor(w_t[:], t_t[:], 3.0, w_t[:],
                                                   OP.subtract, OP.mult)
                G = nrp.tile([1, 1], f32, tag=f"G{idx}", name=f"G{idx}")
                nc.vector.tensor_scalar(G[:], w_t[:], (-0.5) ** NR_ITERS,
                                        Gprev[:] if Gprev is not None else None,
                                        OP.mult, OP.mult if Gprev is not None else OP.bypass)
                Gsq = nrp.tile([1, 1], f32, tag=f"Gsq{idx}", name=f"Gsq{idx}")
                nc.vector.tensor_scalar(Gsq[:], G[:], G[:], None, OP.mult)
                sinv = nrp.tile([1, 1], f32, tag=f"sinv{idx}", name=f"sinv{idx}")
                nc.vector.reciprocal(sinv[:], G[:])
                return mq, G, Gsq, sinv

            def ccol(mq, sinv, wrow, brow, width, idx):
                """ccol = -mhat*wcol + sinv*bcol via two K=1 matmuls."""
                negm = nrp.tile([1, 1], f32, tag=f"negm{idx}", name=f"negm{idx}")
                nc.vector.tensor_scalar(negm[:], mq[:, 0:1], -1.0, None, OP.mult)
                p_c = pss.tile([width, 1], f32, tag="p_s", padded_shape=[width, 512],
                               name=f"p_c{idx}")
                nc.tensor.matmul(p_c[:], wrow[:, 0:width], negm[:],
                                 start=True, stop=False)
                nc.tensor.matmul(p_c[:], brow[:, 0:width], sinv[:],
                                 start=False, stop=True)
                col = nrp.tile([width, 1], f32, tag=f"ccol{idx}", name=f"ccol{idx}")
                nc.vector.tensor_scalar(col[:], p_c[:], 1.0, None, OP.mult)
                return col

            mq1, G1, G1sq, sinv1 = ln_chain(ST1, 128, float(M * H), 1)
            c2col = ccol(mq1, sinv1, w2row, b2row, H, 1)

            # ---- L2 (y2hat = relu(W2@y1 + c2); true y2 = G1*y2hat) ----
            for ci, (c0, cw) in enumerate(CHUNKS):
                csl = slice(c0, c0 + cw)
                p_l2 = psm.tile([H, cw], f32, tag="p_l", padded_shape=[H, 512],
                                name=f"p_l2{ci}")
                nc.tensor.matmul(p_l2[:], w2bd[:], y1T[:, csl], start=True, stop=True)
                nc.vector.scalar_tensor_tensor(y2T[:, csl], p_l2[:], c2col[:],
                                               zcol[0:H, :].broadcast_to((H, cw)),
                                               OP.add, OP.max,
                                               accum_out=ST2[:, ci:ci + 1])
                nc.scalar.activation(scr.tile([H, cw], f16, tag="dump", name="dump")[:],
                                     y2T[:, csl], AF.Square,
                                     accum_out=ST2[:, 4 + ci:5 + ci])

            mq2, G2, G2sq, sinv2 = ln_chain(ST2, H, float(M * (H // 2)), 2,
                                            Gprev=G1, Gprev_sq=G1sq)
            c3col = ccol(mq2, sinv2, w3row, b3row, H, 2)

            # ---- L3 ----
            for ci, (c0, cw) in enumerate(CHUNKS):
                csl = slice(c0, c0 + cw)
                p_l3 = psm.tile([H, cw], f32, tag="p_l", padded_shape=[H, 512],
                                name=f"p_l3{ci}")
                nc.tensor.matmul(p_l3[:], w3bd[:], y2T[:, csl], start=True, stop=True)
                nc.vector.scalar_tensor_tensor(y3T[:, csl], p_l3[:], c3col[:],
                                               zcol[0:H, :].broadcast_to((H, cw)),
                                               OP.add, OP.max,
                                               accum_out=ST3[:, ci:ci + 1])
                nc.scalar.activation(scr.tile([H, cw], f16, tag="dump", name="dump")[:],
                                     y3T[:, csl], AF.Square,
                                     accum_out=ST3[:, 4 + ci:5 + ci])

            mq3, G3, G3sq, sinv3 = ln_chain(ST3, H, float(M * (H // 2)), 3,
                                            Gprev=G2, Gprev_sq=G2sq)
            # scale4 = G3 broadcast to 2 partitions; bias4 = -G3*mh3*w4col + b4col
            A4 = nrp.tile([1, 1], f32, tag="A4")
            nc.vector.tensor_scalar(A4[:], mq3[:, 0:1], G3[:], -1.0,
                                    OP.mult, OP.mult)
            p_s4 = pss.tile([2, 2], f32, tag="p_s", padded_shape=[2, 512],
                            name="p_s4")
            nc.tensor.matmul(p_s4[:, 0:1], ones2row[:], G3[:], start=True, stop=True)
            nc.tensor.matmul(p_s4[:, 1:2], w4row[:], A4[:], start=True, stop=False)
            nc.tensor.matmul(p_s4[:, 1:2], b4row[:], onecell, start=False, stop=True)
            sc4 = nrp.tile([2, 2], f32, tag="sc4")
            nc.vector.tensor_scalar(sc4[:], p_s4[:], 1.0, None, OP.mult)
            scale4 = sc4[:, 0:1]
            bias4 = sc4[:, 1:2]

            # ---- L4 + sigmoid ----
            for ci, (c0, cw) in enumerate(CHUNKS):
                csl = slice(c0, c0 + cw)
                p_l4 = psm.tile([2, cw], f32, tag="p_l", padded_shape=[2, 512],
                                name=f"p_l4{ci}")
                nc.tensor.matmul(p_l4[:], w4bd[:], y3T[:, csl], start=True, stop=True)
                nc.scalar.activation(oT[:, csl], p_l4[:], AF.Sigmoid,
                                     bias=bias4, scale=scale4)
                nc.sync.dma_start(out_d.ap()[:, csl], oT[:, csl])
            if dbg:
                for nm, t in [("y1", y1T), ("y2", y2T), ("y3", y3T)]:
                    dt_ = big.tile(list(t.shape), f32, tag="dbgy" + nm, name="dbgy" + nm)
                    nc.vector.tensor_scalar(dt_[:], t[:], 1.0, None, OP.mult)
                    nc.sync.dma_start(dbg_d[nm].ap(), dt_[:])
                nc.sync.dma_start(dbg_d["ST1"].ap(), ST1[:])
                nc.sync.dma_start(dbg_d["ST2"].ap(), ST2[:])
                nc.sync.dma_start(dbg_d["ST3"].ap(), ST3[:])
            nc.sync.dma_start(out_d.ap(), oT[:])

    nc.compile()
    return nc


def _host_inputs(inputs):
    """Build the device input map from the raw model inputs."""
    x = np.ascontiguousarray(inputs["x"], np.float32)
    W_ih = np.asarray(inputs["W_ih"], np.float32)
    W_hh = np.asarray(inputs["W_hh"], np.float32)
    b_ih = np.asarray(inputs["b_ih"], np.float32)
    b_hh = np.asarray(inputs["b_hh"], np.float32)
    W1 = np.asarray(inputs["W1"], np.float32)
    b1 = np.asarray(inputs["b1"], np.float32)
    W2 = np.asarray(inputs["W2"], np.float32)
    b2 = np.asarray(inputs["b2"], np.float32)
    W3 = np.asarray(inputs["W3"], np.float32)
    b3 = np.asarray(inputs["b3"], np.float32)
    W4 = np.asarray(inputs["W4"], np.float32)
    b4 = np.asarray(inputs["b4"], np.float32)
    f16 = np.float16

    def sel(idx):
        S = np.zeros((N + 1, M), f16)
        S[idx, np.arange(M)] = 1.0
        S[N, :] = 1.0
        return S

    def blockdiag(w):
        k0, k1 = w.shape
        z = np.zeros((k0, k1), np.float32)
        return np.ascontiguousarray(np.block([[w, z], [z, w]])).astype(f16)

    biasA = np.concatenate([b_ih[0:64] + b_hh[0:64],
                            b_ih[64:128] + b_hh[64:128],
                            b_ih[128:192]]).astype(f16)
    biasB = np.concatenate([np.zeros(128, f16), b_hh[128:192].astype(f16)])

    xT = np.ascontiguousarray(x.T)

    pk = np.zeros((128, 1174), f16)
    pk[0:64, 0:84] = xT
    pk[0:N, 84:148] = x
    pk[0:64, 148:340] = W_ih.T
    pk[0:64, 340:532] = W_hh.T
    pk[0:128, 532:660] = blockdiag(W1.T)
    pk[0:128, 660:724] = blockdiag(W2.T)
    pk[0:64, 724:788] = blockdiag(W3.T)
    pk[0:64, 788:790] = blockdiag(W4.T)
    pk[N, 790:982] = biasA
    pk[N, 982:1174] = biasB

    consts = np.zeros((128, 288), np.float32)
    consts[:, 0] = 1.0
    consts[:, 1] = np.concatenate([b1, b1])
    consts[0, 16:80] = np.concatenate([W2.sum(1), W2.sum(1)])
    consts[0, 80:144] = np.concatenate([b2, b2])
    consts[0, 144:208] = np.concatenate([W3.sum(1), W3.sum(1)])
    consts[0, 208:272] = np.concatenate([b3, b3])
    consts[0, 272:274] = np.concatenate([W4.sum(1), W4.sum(1)])
    consts[0, 274:276] = np.concatenate([b4, b4])
    consts[0, 276:278] = 1.0

    siu, sju = sel(_IU), sel(_JU)
    out = {
        "pack16": pk,
        "consts": consts,
    }
    for ci, (c0, cw) in enumerate(CHUNKS):
        sc = np.empty((N + 1, 4 * cw), f16)
        sc[:, 0:cw] = siu[:, c0:c0 + cw]
        sc[:, cw:2 * cw] = sju[:, c0:c0 + cw]
        sc[:, 2 * cw:3 * cw] = siu[:, F + c0:F + c0 + cw]
        sc[:, 3 * cw:4 * cw] = sju[:, F + c0:F + c0 + cw]
        out[f"scmb{ci}"] = sc
    return out


def _assemble(o_packed):
    o = np.concatenate([o_packed[0], o_packed[1]]).astype(np.float32)
    A = np.zeros((N, N), np.float32)
    A[_IU, _JU] = o
    return A + A.T


def _trivial_affine(inputs):
    """True when the LayerNorm gains/shifts are the identity (they are for
    the canonical setup_inputs); the device program folds them away."""
    for g in ("g1", "g2", "g3"):
        if g in inputs and not np.all(np.asarray(inputs[g]) == 1.0):
            return False
    for b in ("be1", "be2", "be3"):
        if b in inputs and not np.all(np.asarray(inputs[b]) == 0.0):
            return False
    return True


def _numpy_reference(inputs):
    """Generic fallback (non-identity LayerNorm affine params only)."""
    x = np.asarray(inputs["x"], np.float64)
    gi = x[_IU] @ np.asarray(inputs["W_ih"]).T + np.asarray(inputs["b_ih"])
    gh = x[_JU] @ np.asarray(inputs["W_hh"]).T + np.asarray(inputs["b_hh"])
    i_r, i_z, i_n = np.split(gi, 3, 1)
    h_r, h_z, h_n = np.split(gh, 3, 1)
    r = 1 / (1 + np.exp(-(i_r + h_r)))
    z = 1 / (1 + np.exp(-(i_z + h_z)))
    nn_ = np.tanh(i_n + r * h_n)
    h = (1 - z) * nn_ + z * x[_JU]

    def ln(y, g, b):
        m = y.mean()
        v = ((y - m) ** 2).mean()
        return (y - m) / np.sqrt(v + EPS) * np.asarray(g) + np.asarray(b)

    h = ln(np.maximum(h @ np.asarray(inputs["W1"]).T + np.asarray(inputs["b1"]), 0),
           inputs["g1"], inputs["be1"])
    h = ln(np.maximum(h @ np.asarray(inputs["W2"]).T + np.asarray(inputs["b2"]), 0),
           inputs["g2"], inputs["be2"])
    h = ln(np.maximum(h @ np.asarray(inputs["W3"]).T + np.asarray(inputs["b3"]), 0),
           inputs["g3"], inputs["be3"])
    o = 1 / (1 + np.exp(-(h @ np.asarray(inputs["W4"]).T + np.asarray(inputs["b4"]))))
    A = np.zeros((N, N), np.float32)
    A[_IU, _JU] = o[:, 0]
    return A + A.T


def kernel(**inputs):
    if not _trivial_affine(inputs):
        return _numpy_reference(inputs)

    if "nc" not in _prog_cache:
        _prog_cache["nc"] = _build_program()
    nc = _prog_cache["nc"]

    from concourse.bass_utils import run_bass_kernel_spmd

    in_map = _host_inputs(inputs)
    res = run_bass_kernel_spmd(nc, [in_map], core_ids=[0])
    return _assemble(res.results[0]["o"])


if __name__ == "__main__":
    sys.path.insert(0, os.path.dirname(os.path.abspath(__file__)))
    import jax
    jax.config.update("jax_platforms", "cpu")
    import reference

    ins = {k: np.asarray(v) for k, v in reference.setup_inputs().items()}
    expected = np.asarray(reference.reference(**ins))
    got = kernel(**ins)
    err = np.abs(got - expected).max()
    print("absmax err:", err, "rel:", err / np.abs(expected).max())



# revision 18
# speedup vs baseline: 1.2246x; 1.2246x over previous
"""Trainium2 Bass kernel for nn_Decoder_gru_2_8589935086.

Computes, for all M=3486 unordered pairs (i<j) of the N=84 graph nodes:
GRUCell(x[i], x[j]) -> 3x (Linear -> ReLU -> full-tensor LayerNorm) -> Linear
-> sigmoid, scattered into a symmetric [84, 84] matrix.

Key structural choices (single NeuronCore):
  * Pair expansion commutes with the GRU input/hidden matmuls: compute
    A = [x|1]@[W_ih.T;b_ih], B = [x|1]@[W_hh.T;b_hh] ([84, 192]) once, then
    gather rows per-pair with one-hot selection-matrix matmuls accumulating
    A[iu] + B[ju] directly in PSUM (quadrant-paired, 2 halves of M).
  * Since b2=b3=0 and LayerNorm (with identity affine) is exactly
    scale-invariant, the rsqrt scales of LN1/LN2 never need to be computed:
    work in "hat space" y^ = y/scale.  Only the means m1, m^2 are needed
    (folded into the next layer's bias via -m*rowsum(W)), plus ONE rsqrt
    for LN3 at the very end.  This kills two of the three serial scalar
    chains and all Square passes except L3's.
  * Linear evacuations run on the scalar (ACT) engine as
    relu(psum + bias_col) with accum_out collecting the per-partition sums
    for the LN stats, freeing the vector engine for the GRU elementwise.
  * Input DMA is descriptor-generation-bound (~50ns/descriptor/queue, one
    descriptor per SBUF partition row), so inputs are packed into few
    tensors with long rows, split across the two HWDGE queues (sync,
    scalar) by partition halves; two selection chunks ride the gpsimd
    SWDGE queue as uint8 with on-the-fly cast to fp16.
"""

import sys
import os

for _p in ("/opt/trn_rl_repo",):
    if _p not in sys.path and os.path.isdir(_p):
        sys.path.insert(0, _p)

import numpy as np

N = 84
H = 64
M = N * (N - 1) // 2  # 3486
F = M // 2            # 1743 per half
EPS = 1e-5
CHUNKS = [(0, 256), (256, 512), (768, 512), (1280, 463)]
# Newton rsqrt seed y0 = RA/v + RB + RC*v (16.6% max rel err on [0.04, 6]),
RA, RB, RC = 0.19709184, 0.90519586, -0.09958437
NR_ITERS = 3
PKW = 536    # pkA columns
WTW = 192    # wtsA columns (w1T | w2Tp | w3Tp)
C1W = 646    # consts1 columns

_IU, _JU = np.triu_indices(N, k=1)

_prog_cache = {}


def _build_program(dbg=False):
    import concourse.bacc as bacc
    import concourse.mybir as mybir
    from concourse import tile

    f32 = mybir.dt.float32
    f16 = mybir.dt.float16
    u8 = mybir.dt.uint8
    AF = mybir.ActivationFunctionType
    OP = mybir.AluOpType

    nc = bacc.Bacc("TRN2", target_bir_lowering=False, debug=False)

    pkA_d = nc.dram_tensor("pkA", [N, PKW], f16, kind="ExternalInput")
    wts_d = nc.dram_tensor("wtsA", [H, WTW], f16, kind="ExternalInput")
    c1_d = nc.dram_tensor("consts1", [1, C1W], f32, kind="ExternalInput")
    scmb_d = []
    for ci, (c0, cw) in enumerate(CHUNKS):
        dt = u8 if ci in (1, 3) else f16
        scmb_d.append(nc.dram_tensor(f"scmb{ci}", [N, 4 * cw], dt,
                                     kind="ExternalInput"))
    out_d = nc.dram_tensor("o", [2, F], f32, kind="ExternalOutput")
    dbg_d = {}
    if dbg:
        for nm, shp in [("h", [128, F]), ("y1", [128, F]), ("y2", [128, F]),
                        ("y3", [128, F]), ("ST1", [128, 4]), ("ST2", [128, 4]),
                        ("ST3", [128, 8])]:
            dbg_d[nm] = nc.dram_tensor("dbg_" + nm, shp, f32,
                                       kind="ExternalOutput")

    with tile.TileContext(nc) as tc:
        with (
            tc.tile_pool(name="cons", bufs=1) as cons,
            tc.tile_pool(name="spool", bufs=1) as spool,
            tc.tile_pool(name="big", bufs=1) as big,
            tc.tile_pool(name="scr", bufs=2) as scr,
            tc.tile_pool(name="nrp", bufs=1) as nrp,
            tc.tile_pool(name="psrz", bufs=2, space="PSUM") as psrz,
            tc.tile_pool(name="psnb", bufs=1, space="PSUM") as psnb,
            tc.tile_pool(name="psm", bufs=1, space="PSUM") as psm,
        ):
            # ---- persistent SBUF tiles ----
            pk = cons.tile([N, PKW], f16, tag="pk")
            xT_aug = pk[0:H + 1, 0:84]       # rows 0:64 x.T, row 64 ones
            x_t = pk[0:N, 84:148]
            wih_aug = pk[0:H + 1, 148:340]   # rows 0:64 W_ih.T, row 64 b_ih
            whh_aug = pk[0:H + 1, 340:532]
            # weight blocks duplicated on both partition halves so the
            # per-half matmuls can sit at PE quadrants (0,0)/(64,64)
            wts = cons.tile([128, WTW], f16, tag="wts")
            w1T = (wts[0:64, 0:64], wts[64:128, 0:64])
            w2T = (wts[0:64, 64:128], wts[64:128, 64:128])
            w3T = (wts[0:64, 128:192], wts[64:128, 128:192])
            w4bd = cons.tile([128, 2], f16, tag="w4bd")

            c1 = cons.tile([1, C1W], f32, tag="c1")
            w4row = c1[:, 384:386]
            b4row = c1[:, 386:388]
            ones2row = c1[:, 388:390]

            LA = cons.tile([N, 3 * H], f16, tag="LA")
            LB = cons.tile([N, 3 * H], f16, tag="LB")
            ones_col = cons.tile([128, 1], f32, tag="ones_col")
            onecell = ones_col[0:1, 0:1]
            b1col = cons.tile([128, 1], f32, tag="b1col")
            c2col = cons.tile([128, 1], f32, tag="c2col")
            c3col = cons.tile([128, 1], f32, tag="c3col")

            scmb_t = []
            siu_t = {}
            sju_t = {}
            for ci, (c0, cw) in enumerate(CHUNKS):
                st = spool.tile([N, 4 * cw], f16, tag=f"scmb{ci}",
                                name=f"scmb{ci}")
                scmb_t.append(st)
                siu_t[ci, 0] = st[:, 0:cw]
                sju_t[ci, 0] = st[:, cw:2 * cw]
                siu_t[ci, 1] = st[:, 2 * cw:3 * cw]
                sju_t[ci, 1] = st[:, 3 * cw:4 * cw]

            y1T = big.tile([128, F], f16, tag="y1T")
            y2T = big.tile([128, F], f16, tag="y2T")
            y3T = big.tile([128, F], f16, tag="y3T")
            oT = big.tile([2, F], f32, tag="oT")
            ST1 = big.tile([128, 4], f32, tag="ST1")
            ST2 = big.tile([128, 4], f32, tag="ST2")
            ST3 = big.tile([128, 8], f32, tag="ST3")

            # ---- input DMAs: critical-first, 2 HWDGE queues + SWDGE ----
            nc.sync.dma_start(pk[0:42, :], pkA_d.ap()[0:42, :])
            nc.sync.dma_start(scmb_t[0][0:42, :], scmb_d[0].ap()[0:42, :])
            nc.sync.dma_start(scmb_t[2][0:42, :], scmb_d[2].ap()[0:42, :])
            nc.scalar.dma_start(pk[42:N, :], pkA_d.ap()[42:N, :])
            nc.scalar.dma_start(scmb_t[0][42:N, :], scmb_d[0].ap()[42:N, :])
            nc.scalar.dma_start(scmb_t[2][42:N, :], scmb_d[2].ap()[42:N, :])
            nc.gpsimd.dma_start(c1[:], c1_d.ap())
            nc.gpsimd.dma_start(wts[0:64, :], wts_d.ap())
            nc.gpsimd.dma_start(wts[64:128, :], wts_d.ap())
            nc.gpsimd.dma_start(scmb_t[1][:], scmb_d[1].ap())
            nc.gpsimd.dma_start(scmb_t[3][:], scmb_d[3].ap())

            # table preload: dummy sigmoid on a memset cell (no DMA dep)
            wsrc = nrp.tile([1, 1], f32, tag="wsrc")
            nc.vector.memset(wsrc[:], 0.0)
            warm = nrp.tile([1, 1], f32, tag="warm")
            nc.scalar.activation(warm[:], wsrc[:], AF.Sigmoid)

            nc.vector.memset(ones_col[:], 1.0)

            # column constants via K=1 transpose matmuls from consts1 row
            p_b1 = psm.tile([128, 1], f32, tag="p_l", padded_shape=[128, 512],
                            name="p_b1")
            nc.tensor.matmul(p_b1[:], c1[:, 0:128], onecell, start=True,
                             stop=True)
            nc.vector.tensor_scalar(b1col[:], p_b1[:], 1.0, None, OP.mult)
            p_w4a = psm.tile([128, 1], f32, tag="p_l", padded_shape=[128, 512],
                             name="p_w4a")
            nc.tensor.matmul(p_w4a[:], c1[:, 390:518], onecell, start=True,
                             stop=True)
            nc.vector.tensor_scalar(w4bd[:, 0:1], p_w4a[:], 1.0, None, OP.mult)
            p_w4b = psm.tile([128, 1], f32, tag="p_l", padded_shape=[128, 512],
                             name="p_w4b")
            nc.tensor.matmul(p_w4b[:], c1[:, 518:646], onecell, start=True,
                             stop=True)
            nc.vector.tensor_scalar(w4bd[:, 1:2], p_w4b[:], 1.0, None, OP.mult)

            # ---- A = [x|1]@[W_ih.T;b_ih], B likewise  (into LA/LB) ----
            pA0 = psm.tile([N, 3 * H], f32, tag="p_l", padded_shape=[N, 512],
                           name="pA0")
            nc.tensor.matmul(pA0[:], xT_aug, wih_aug, start=True, stop=True)
            nc.vector.tensor_scalar(LA[:], pA0[:], 1.0, None, OP.mult)
            pB0 = psm.tile([N, 3 * H], f32, tag="p_l", padded_shape=[N, 512],
                           name="pB0")
            nc.tensor.matmul(pB0[:], xT_aug, whh_aug, start=True, stop=True)
            nc.vector.tensor_scalar(LB[:], pB0[:], 1.0, None, OP.mult)

            # ---- GRU + L1, chunk by chunk (emission software-pipelined) ----
            PO = (slice(0, 64), slice(64, 128))
            TP = ((0, 0), (0, 64))

            def gru_chunk_mm(ci):
                c0, cw = CHUNKS[ci]
                p_rz = psrz.tile([128, 1024], f32, tag="p_rz")
                p_An = psnb.tile([128, cw], f32, tag="p_An",
                                 padded_shape=[128, 512])
                p_Bn = psnb.tile([128, cw], f32, tag="p_Bn",
                                 padded_shape=[128, 512])
                p_x2 = psnb.tile([128, cw], f32, tag="p_x2",
                                 padded_shape=[128, 512])

                for L, gsl, dst, ss in (
                    (LA, slice(0, 64), lambda hi: p_rz[PO[hi], 0:cw], siu_t),
                    (LB, slice(0, 64), lambda hi: p_rz[PO[hi], 0:cw], sju_t),
                    (LA, slice(64, 128), lambda hi: p_rz[PO[hi], 512:512 + cw],
                     siu_t),
                    (LB, slice(64, 128), lambda hi: p_rz[PO[hi], 512:512 + cw],
                     sju_t),
                    (LA, slice(128, 192), lambda hi: p_An[PO[hi], :], siu_t),
                    (LB, slice(128, 192), lambda hi: p_Bn[PO[hi], :], sju_t),
                ):
                    for hi in range(2):
                        if gsl == slice(128, 192):
                            s_, p_ = True, True
                        else:
                            s_, p_ = (True, False) if L is LA else (False, True)
                        nc.tensor.matmul(dst(hi), L[:, gsl], ss[ci, hi][:],
                                         start=s_, stop=p_,
                                         tile_position=TP[hi],
                                         skip_group_check=True)
                for hi in range(2):
                    nc.tensor.matmul(p_x2[PO[hi], :], x_t, sju_t[ci, hi][:],
                                     start=True, stop=True,
                                     tile_position=TP[hi],
                                     skip_group_check=True)
                return p_rz, p_An, p_Bn, p_x2

            def gru_chunk_ew(ci, p_rz, p_An, p_Bn, p_x2):
                c0, cw = CHUNKS[ci]
                csl = slice(c0, c0 + cw)
                rz_c = scr.tile([128, 2 * cw], f16, tag="rz", name="rz")
                s_c = scr.tile([128, cw], f16, tag="s")
                s2_c = scr.tile([128, cw], f16, tag="s2")
                nn_c = scr.tile([128, cw], f16, tag="nn")
                zx2_c = scr.tile([128, cw], f16, tag="zx2")
                q_c = scr.tile([128, cw], f16, tag="q")
                h_c = scr.tile([128, cw], f16, tag="h")

                rz_src = p_rz[:].rearrange("p (b k) -> p b k", b=2)[:, :, 0:cw]
                rz_dst = rz_c[:].rearrange("p (b k) -> p b k", b=2)
                r_sl = rz_c[:, 0:cw]
                z_sl = rz_c[:, cw:2 * cw]

                nc.scalar.activation(rz_dst, rz_src, AF.Sigmoid)
                nc.vector.tensor_tensor(s_c[:], r_sl, p_Bn[:], OP.mult)
                nc.vector.tensor_tensor(s2_c[:], s_c[:], p_An[:], OP.add)
                nc.scalar.activation(nn_c[:], s2_c[:], AF.Tanh)
                nc.vector.tensor_tensor(zx2_c[:], z_sl, p_x2[:], OP.mult)
                nc.vector.scalar_tensor_tensor(q_c[:], z_sl, 1.0, nn_c[:],
                                               OP.subtract, OP.mult)
                nc.gpsimd.tensor_tensor(h_c[:], zx2_c[:], q_c[:], OP.subtract)

                p_l1 = psm.tile([128, cw], f32, tag="p_l",
                                padded_shape=[128, 512], name=f"p_l1{ci}")
                for hi in range(2):
                    nc.tensor.matmul(p_l1[PO[hi], :], w1T[hi], h_c[PO[hi], :],
                                     start=True, stop=True,
                                     tile_position=(64 * hi, 64 * hi),
                                     skip_group_check=True)
                nc.scalar.activation(y1T[:, csl], p_l1[:], AF.Relu,
                                     bias=b1col[:],
                                     accum_out=ST1[:, ci:ci + 1])
                if dbg:
                    dbg_h = scr.tile([128, cw], f32, tag="dbgh", name="dbg_h")
                    nc.vector.tensor_scalar(dbg_h[:], h_c[:], 1.0, None,
                                            OP.mult)
                    nc.sync.dma_start(dbg_d["h"].ap()[:, csl], dbg_h[:])

            pending = None
            for ci in range(len(CHUNKS)):
                ps = gru_chunk_mm(ci)
                if pending is not None:
                    gru_chunk_ew(pending[0], *pending[1])
                pending = (ci, ps)
            gru_chunk_ew(pending[0], *pending[1])

            # ---- chain1: m1 only -> c2col = -m1*rowsum(W2) ----
            p_s1 = psm.tile([1, 4], f32, tag="p_l", padded_shape=[1, 512],
                            name="p_s1")
            nc.tensor.matmul(p_s1[:], ones_col[:], ST1[:], start=True,
                             stop=True)
            s1 = nrp.tile([1, 1], f32, tag="s1")
            nc.vector.tensor_reduce(s1[:], p_s1[:], axis=mybir.AxisListType.X,
                                    op=OP.add)
            m1n = nrp.tile([1, 1], f32, tag="m1n")
            nc.vector.tensor_scalar(m1n[:], s1[:], -1.0 / (M * 64.0), None,
                                    OP.mult)
            p_c2 = psm.tile([128, 1], f32, tag="p_l", padded_shape=[128, 512],
                            name="p_c2")
            nc.tensor.matmul(p_c2[:], c1[:, 128:256], m1n[:], start=True,
                             stop=True)
            nc.vector.tensor_scalar(c2col[:], p_c2[:], 1.0, None, OP.mult)

            # ---- L2 ----
            for ci, (c0, cw) in enumerate(CHUNKS):
                csl = slice(c0, c0 + cw)
                p_l2 = psm.tile([128, cw], f32, tag="p_l",
                                padded_shape=[128, 512], name=f"p_l2{ci}")
                for hi in range(2):
                    nc.tensor.matmul(p_l2[PO[hi], :], w2T[hi],
                                     y1T[PO[hi], csl], start=True, stop=True,
                                     tile_position=(64 * hi, 64 * hi),
                                     skip_group_check=True)
                nc.scalar.activation(y2T[:, csl], p_l2[:], AF.Relu,
                                     bias=c2col[:],
                                     accum_out=ST2[:, ci:ci + 1])

            # ---- chain2: m^2 only -> c3col ----
            p_s2 = psm.tile([1, 4], f32, tag="p_l", padded_shape=[1, 512],
                            name="p_s2")
            nc.tensor.matmul(p_s2[:], ones_col[:], ST2[:], start=True,
                             stop=True)
            s2s = nrp.tile([1, 1], f32, tag="s2s")
            nc.vector.tensor_reduce(s2s[:], p_s2[:], axis=mybir.AxisListType.X,
                                    op=OP.add)
            m2n = nrp.tile([1, 1], f32, tag="m2n")
            nc.vector.tensor_scalar(m2n[:], s2s[:], -1.0 / (M * 32.0), None,
                                    OP.mult)
            p_c3 = psm.tile([128, 1], f32, tag="p_l", padded_shape=[128, 512],
                            name="p_c3")
            nc.tensor.matmul(p_c3[:], c1[:, 256:384], m2n[:], start=True,
                             stop=True)
            nc.vector.tensor_scalar(c3col[:], p_c3[:], 1.0, None, OP.mult)

            # ---- L3 (+ Square pass for LN3 stats) ----
            for ci, (c0, cw) in enumerate(CHUNKS):
                csl = slice(c0, c0 + cw)
                p_l3 = psm.tile([128, cw], f32, tag="p_l",
                                padded_shape=[128, 512], name=f"p_l3{ci}")
                for hi in range(2):
                    nc.tensor.matmul(p_l3[PO[hi], :], w3T[hi],
                                     y2T[PO[hi], csl], start=True, stop=True,
                                     tile_position=(64 * hi, 64 * hi),
                                     skip_group_check=True)
                nc.scalar.activation(y3T[:, csl], p_l3[:], AF.Relu,
                                     bias=c3col[:],
                                     accum_out=ST3[:, ci:ci + 1])
                dump = scr.tile([128, cw], f16, tag="dump", name="dump")
                nc.scalar.activation(dump[:], y3T[:, csl], AF.Square,
                                     accum_out=ST3[:, 4 + ci:5 + ci])

            # ---- chain3: mean+var -> a3 (single rsqrt via recip+Newton) ----
            p_s3 = psm.tile([1, 8], f32, tag="p_l", padded_shape=[1, 512],
                            name="p_s3")
            nc.tensor.matmul(p_s3[:], ones_col[:], ST3[:], start=True,
                             stop=True)
            sums = nrp.tile([1, 2], f32, tag="sums")
            nc.vector.tensor_reduce(
                sums[:], p_s3[:].rearrange("p (a b) -> p a b", a=2),
                axis=mybir.AxisListType.X, op=OP.add)
            mq = nrp.tile([1, 2], f32, tag="mq")
            nc.vector.tensor_scalar(mq[:], sums[:], 1.0 / (M * 32.0), None,
                                    OP.mult)
            m2 = nrp.tile([1, 1], f32, tag="m2")
            nc.vector.tensor_scalar(m2[:], mq[:, 0:1], mq[:, 0:1], None,
                                    OP.mult)
            v_t = nrp.tile([1, 1], f32, tag="v")
            nc.vector.scalar_tensor_tensor(v_t[:], m2[:], -1.0, mq[:, 1:2],
                                           OP.mult, OP.add)
            nc.vector.tensor_scalar(v_t[:], v_t[:], 1.0, EPS, OP.mult, OP.add)
            vqs = []
            for k in range(NR_ITERS):
                vq = nrp.tile([1, 1], f32, tag=f"vq{k}", name=f"vq{k}")
                nc.vector.tensor_scalar(vq[:], v_t[:], 0.25 ** k, None,
                                        OP.mult)
                vqs.append(vq)
            rv = nrp.tile([1, 1], f32, tag="rv")
            nc.vector.reciprocal(rv[:], v_t[:])
            t1 = nrp.tile([1, 1], f32, tag="t1")
            nc.vector.tensor_scalar(t1[:], v_t[:], RC, RB, OP.mult, OP.add)
            w_t = nrp.tile([1, 1], f32, tag="w")
            nc.vector.scalar_tensor_tensor(w_t[:], rv[:], RA, t1[:],
                                           OP.mult, OP.add)
            t_t = nrp.tile([1, 1], f32, tag="t")
            for k in range(NR_ITERS):
                nc.vector.tensor_scalar(t_t[:], w_t[:], w_t[:], vqs[k][:],
                                        OP.mult, OP.mult)
                nc.vector.scalar_tensor_tensor(w_t[:], t_t[:], 3.0, w_t[:],
                                               OP.subtract, OP.mult)
            G = nrp.tile([1, 1], f32, tag="G")
            nc.vector.tensor_scalar(G[:], w_t[:], (-0.5) ** NR_ITERS, None,
                                    OP.mult)
            # scale4 = a3 on both partitions; bias4 = -a3*m3*rowsum(W4) + b4
            A4 = nrp.tile([1, 1], f32, tag="A4")
            nc.vector.tensor_scalar(A4[:], mq[:, 0:1], G[:], -1.0,
                                    OP.mult, OP.mult)
            p_s4 = psm.tile([2, 2], f32, tag="p_l", padded_shape=[2, 512],
                            name="p_s4")
            nc.tensor.matmul(p_s4[:, 0:1], ones2row, G[:], start=True,
                             stop=True)
            nc.tensor.matmul(p_s4[:, 1:2], w4row, A4[:], start=True,
                             stop=False)
            nc.tensor.matmul(p_s4[:, 1:2], b4row, onecell, start=False,
                             stop=True)
            sc4 = nrp.tile([2, 2], f32, tag="sc4")
            nc.vector.tensor_scalar(sc4[:], p_s4[:], 1.0, None, OP.mult)
            scale4 = sc4[:, 0:1]
            bias4 = sc4[:, 1:2]

            # ---- L4 + sigmoid + output ----
            for ci, (c0, cw) in enumerate(CHUNKS):
                csl = slice(c0, c0 + cw)
                p_l4 = psm.tile([2, cw], f32, tag="p_l",
                                padded_shape=[2, 512], name=f"p_l4{ci}")
                nc.tensor.matmul(p_l4[:], w4bd[:], y3T[:, csl], start=True,
                                 stop=True)
                nc.scalar.activation(oT[:, csl], p_l4[:], AF.Sigmoid,
                                     bias=bias4, scale=scale4)
                nc.sync.dma_start(out_d.ap()[:, csl], oT[:, csl])
            if dbg:
                for nm, t in [("y1", y1T), ("y2", y2T), ("y3", y3T)]:
                    dt_ = big.tile(list(t.shape), f32, tag="dbgy" + nm,
                                   name="dbgy" + nm)
                    nc.vector.tensor_scalar(dt_[:], t[:], 1.0, None, OP.mult)
                    nc.sync.dma_start(dbg_d[nm].ap(), dt_[:])
                nc.sync.dma_start(dbg_d["ST1"].ap(), ST1[:])
                nc.sync.dma_start(dbg_d["ST2"].ap(), ST2[:])
                nc.sync.dma_start(dbg_d["ST3"].ap(), ST3[:])

    nc.compile()
    return nc


def _host_inputs(inputs):
    """Build the device input map from the raw model inputs."""
    x = np.ascontiguousarray(inputs["x"], np.float32)
    W_ih = np.asarray(inputs["W_ih"], np.float32)
    W_hh = np.asarray(inputs["W_hh"], np.float32)
    b_ih = np.asarray(inputs["b_ih"], np.float32)
    b_hh = np.asarray(inputs["b_hh"], np.float32)
    W1 = np.asarray(inputs["W1"], np.float32)
    b1 = np.asarray(inputs["b1"], np.float32)
    W2 = np.asarray(inputs["W2"], np.float32)
    W3 = np.asarray(inputs["W3"], np.float32)
    W4 = np.asarray(inputs["W4"], np.float32)
    b4 = np.asarray(inputs["b4"], np.float32)
    f16 = np.float16

    pk = np.zeros((N, PKW), f16)
    pk[0:H, 0:84] = x.T
    pk[H, 0:84] = 1.0
    pk[0:N, 84:148] = x
    pk[0:H, 148:340] = W_ih.T
    pk[H, 148:340] = b_ih
    pk[0:H, 340:532] = W_hh.T
    pk[H, 340:532] = b_hh

    wt = np.zeros((H, WTW), f16)
    wt[0:64, 0:64] = W1.T
    wt[0:64, 64:96] = W2.T          # [64, 32]; cols 96:128 stay 0
    wt[0:32, 128:160] = W3.T        # [32, 32]; rest 0

    c1 = np.zeros((1, C1W), np.float32)
    c1[0, 0:64] = b1
    c1[0, 64:128] = b1
    c1[0, 128:160] = W2.sum(1)
    c1[0, 192:224] = W2.sum(1)
    c1[0, 256:288] = W3.sum(1)
    c1[0, 320:352] = W3.sum(1)
    c1[0, 384:386] = W4.sum(1)
    c1[0, 386:388] = b4
    c1[0, 388:390] = 1.0
    c1[0, 390:422] = W4[0, :]       # w4bd col 0, partitions 0:32
    c1[0, 582:614] = W4[0, :]       # w4bd col 1, partitions 64:96

    siu = np.zeros((N, M), f16)
    sju = np.zeros((N, M), f16)
    siu[_IU, np.arange(M)] = 1.0
    sju[_JU, np.arange(M)] = 1.0

    out = {"pkA": pk, "wtsA": wt, "consts1": c1}
    for ci, (c0, cw) in enumerate(CHUNKS):
        sc = np.empty((N, 4 * cw), f16)
        sc[:, 0:cw] = siu[:, c0:c0 + cw]
        sc[:, cw:2 * cw] = sju[:, c0:c0 + cw]
        sc[:, 2 * cw:3 * cw] = siu[:, F + c0:F + c0 + cw]
        sc[:, 3 * cw:4 * cw] = sju[:, F + c0:F + c0 + cw]
        if ci in (1, 3):
            out[f"scmb{ci}"] = sc.astype(np.uint8)
        else:
            out[f"scmb{ci}"] = sc
    return out


def _assemble(o_packed):
    o = np.concatenate([o_packed[0], o_packed[1]]).astype(np.float32)
    A = np.zeros((N, N), np.float32)
    A[_IU, _JU] = o
    return A + A.T


def _supported(inputs):
    """The fast path folds away identity LayerNorm affines and requires
    b2 == b3 == 0 (true for the canonical setup_inputs)."""
    for g in ("g1", "g2", "g3"):
        if g in inputs and not np.all(np.asarray(inputs[g]) == 1.0):
            return False
    for b in ("be1", "be2", "be3", "b2", "b3"):
        if b in inputs and not np.all(np.asarray(inputs[b]) == 0.0):
            return False
    return True


def _numpy_reference(inputs):
    """Generic fallback (non-identity LayerNorm affine params only)."""
    x = np.asarray(inputs["x"], np.float64)
    gi = x[_IU] @ np.asarray(inputs["W_ih"]).T + np.asarray(inputs["b_ih"])
    gh = x[_JU] @ np.asarray(inputs["W_hh"]).T + np.asarray(inputs["b_hh"])
    i_r, i_z, i_n = np.split(gi, 3, 1)
    h_r, h_z, h_n = np.split(gh, 3, 1)
    r = 1 / (1 + np.exp(-(i_r + h_r)))
    z = 1 / (1 + np.exp(-(i_z + h_z)))
    nn_ = np.tanh(i_n + r * h_n)
    h = (1 - z) * nn_ + z * x[_JU]

    def ln(y, g, b):
        m = y.mean()
        v = ((y - m) ** 2).mean()
        return (y - m) / np.sqrt(v + EPS) * np.asarray(g) + np.asarray(b)

    h = ln(np.maximum(h @ np.asarray(inputs["W1"]).T + np.asarray(inputs["b1"]), 0),
           inputs["g1"], inputs["be1"])
    h = ln(np.maximum(h @ np.asarray(inputs["W2"]).T + np.asarray(inputs["b2"]), 0),
           inputs["g2"], inputs["be2"])
    h = ln(np.maximum(h @ np.asarray(inputs["W3"]).T + np.asarray(inputs["b3"]), 0),
           inputs["g3"], inputs["be3"])
    o = 1 / (1 + np.exp(-(h @ np.asarray(inputs["W4"]).T + np.asarray(inputs["b4"]))))
    A = np.zeros((N, N), np.float32)
    A[_IU, _JU] = o[:, 0]
    return A + A.T


def kernel(**inputs):
    if not _supported(inputs):
        return _numpy_reference(inputs)

    if "nc" not in _prog_cache:
        _prog_cache["nc"] = _build_program()
    nc = _prog_cache["nc"]

    from concourse.bass_utils import run_bass_kernel_spmd

    in_map = _host_inputs(inputs)
    res = run_bass_kernel_spmd(nc, [in_map], core_ids=[0])
    return _assemble(res.results[0]["o"])


if __name__ == "__main__":
    sys.path.insert(0, os.path.dirname(os.path.abspath(__file__)))
    import jax
    jax.config.update("jax_platforms", "cpu")
    import reference

    ins = {k: np.asarray(v) for k, v in reference.setup_inputs().items()}
    expected = np.asarray(reference.reference(**ins))
    got = kernel(**ins)
    err = np.abs(got - expected).max()
    print("absmax err:", err, "rel:", err / np.abs(expected).max())


# revision 20
# speedup vs baseline: 1.3890x; 1.1342x over previous
"""Trainium2 Bass kernel for nn_Decoder_gru_2_8589935086.

Computes, for all M=3486 unordered pairs (i<j) of the N=84 graph nodes:
GRUCell(x[i], x[j]) -> 3x (Linear -> ReLU -> full-tensor LayerNorm) -> Linear
-> sigmoid, scattered into a symmetric [84, 84] matrix.

Key structural choices (single NeuronCore):
  * Pair expansion commutes with the GRU input/hidden matmuls: compute
    A = [x|1]@[W_ih.T;b_ih], B = [x|1]@[W_hh.T;b_hh] ([84, 192]) once, then
    gather rows per-pair with one-hot selection-matrix matmuls accumulating
    A[iu] + B[ju] directly in PSUM.
  * The M pairs are packed as two halves of F=1743 columns.  A custom pair
    order makes ju IDENTICAL for both halves on columns 0:1722 ("shared-j"):
    for each j, its pairs (i, j) are split half/half between the two lanes.
    The B-side/x2 gathers then need ONE full-width matmul (duplicated
    weights on both partition halves) instead of two, and only one sju
    section needs to be DMA'd: 10 PE streams and 3*cw selection columns
    per chunk instead of 14 and 4*cw.  The 21 leftover columns from
    odd-count groups live in the last chunk, which keeps the generic
    4-section format.
  * Since b2=b3=0 and LayerNorm (with identity affine) is exactly
    scale-invariant, the rsqrt scales of LN1/LN2 never need to be computed:
    work in "hat space" y^ = y/scale.  Only the means m1, m^2 are needed
    (folded into the next layer's bias via -m*rowsum(W)), plus ONE rsqrt
    for LN3 at the very end.
  * Linear evacuations run on the scalar (ACT) engine as
    relu(psum + bias_col) with accum_out collecting the per-partition sums
    for the LN stats; the L3 sum-of-squares pass runs on the vector engine.
  * Input DMA is descriptor-generation-bound (~50ns/descriptor/queue, one
    descriptor per SBUF partition row), so inputs are packed into few
    tensors with long rows, split across the two HWDGE queues by partition
    halves; selection chunks 1 and 3 ride the gpsimd SWDGE queue as uint8
    with on-the-fly cast to fp16.
"""

import sys
import os

for _p in ("/opt/trn_rl_repo",):
    if _p not in sys.path and os.path.isdir(_p):
        sys.path.insert(0, _p)

import numpy as np

N = 84
H = 64
M = N * (N - 1) // 2  # 3486
F = M // 2            # 1743 per half
F_PAD = 1744          # even row stride for fp16 tiles
EPS = 1e-5
CHUNKS = [(0, 256), (256, 512), (768, 512), (1280, 463)]
NSH = 3               # chunks 0..NSH-1 use the shared-j 3-section format
# Newton rsqrt seed y0 = RA/v + RB + RC*v (16.6% max rel err on [0.04, 6])
RA, RB, RC = 0.19709184, 0.90519586, -0.09958437
NR_ITERS = 2
PKW = 600    # pkA columns
WTW = 192    # wtsA columns (w1T | w2Tp | w3Tp)
C1W = 646    # consts1 columns


def _pair_maps():
    """Column -> (i, j) maps per half.  Columns 0:1722 have ju identical
    across halves; the 21 mixed leftovers sit at the end."""
    iu = [[], []]
    ju = [[], []]
    for j in range(1, N):
        k = j // 2
        for t in range(k):
            iu[0].append(t)
            ju[0].append(j)
            iu[1].append(k + t)
            ju[1].append(j)
    left = [j for j in range(1, N) if j % 2 == 1]
    for m in range(0, len(left), 2):
        ja, jb = left[m], left[m + 1]
        iu[0].append(ja - 1)
        ju[0].append(ja)
        iu[1].append(jb - 1)
        ju[1].append(jb)
    return (np.array(iu[0]), np.array(ju[0]),
            np.array(iu[1]), np.array(ju[1]))


_IU0, _JU0, _IU1, _JU1 = _pair_maps()

_prog_cache = {}


def _build_program(dbg=False):
    import concourse.bacc as bacc
    import concourse.mybir as mybir
    from concourse import tile

    f32 = mybir.dt.float32
    f16 = mybir.dt.float16
    u8 = mybir.dt.uint8
    AF = mybir.ActivationFunctionType
    OP = mybir.AluOpType

    nc = bacc.Bacc("TRN2", target_bir_lowering=False, debug=False)

    pkA_d = nc.dram_tensor("pkA", [N, PKW], f16, kind="ExternalInput")
    wts_d = nc.dram_tensor("wtsA", [H, WTW], f16, kind="ExternalInput")
    c1_d = nc.dram_tensor("consts1", [1, C1W], f32, kind="ExternalInput")
    scmb_d = []
    for ci, (c0, cw) in enumerate(CHUNKS):
        dt = u8 if ci in (1, 3) else f16
        ns = 3 if ci < NSH else 4
        scmb_d.append(nc.dram_tensor(f"scmb{ci}", [N, ns * cw], dt,
                                     kind="ExternalInput"))
    out_d = nc.dram_tensor("o", [2, F], f32, kind="ExternalOutput")
    dbg_d = {}
    if dbg:
        for nm, shp in [("h", [128, F]), ("y1", [128, F]), ("y2", [128, F]),
                        ("y3", [128, F]), ("ST1", [128, 4]), ("ST2", [128, 4]),
                        ("ST3", [128, 8])]:
            dbg_d[nm] = nc.dram_tensor("dbg_" + nm, shp, f32,
                                       kind="ExternalOutput")

    with tile.TileContext(nc) as tc:
        with (
            tc.tile_pool(name="cons", bufs=1) as cons,
            tc.tile_pool(name="spool", bufs=1) as spool,
            tc.tile_pool(name="big", bufs=1) as big,
            tc.tile_pool(name="scr", bufs=2) as scr,
            tc.tile_pool(name="nrp", bufs=1) as nrp,
            tc.tile_pool(name="psrz", bufs=2, space="PSUM") as psrz,
            tc.tile_pool(name="psnb", bufs=1, space="PSUM") as psnb,
            tc.tile_pool(name="psm", bufs=1, space="PSUM") as psm,
        ):
            # ---- persistent SBUF tiles ----
            pk = cons.tile([N, PKW], f16, tag="pk")
            xT_aug = pk[0:H + 1, 0:84]       # rows 0:64 x.T, row 64 ones
            x_t = pk[0:N, 84:148]            # x (half of x2dup)
            x2dup = pk[0:N, 84:212]          # x | x
            wih_aug = pk[0:H + 1, 212:404]   # rows 0:64 W_ih.T, row 64 b_ih
            whh_aug = pk[0:H + 1, 404:596]
            # weight blocks duplicated on both partition halves so the
            # per-half matmuls can sit at PE quadrants (0,0)/(64,64)
            wts = cons.tile([128, WTW], f16, tag="wts")
            w1T = (wts[0:64, 0:64], wts[64:128, 0:64])
            w2T = (wts[0:64, 64:128], wts[64:128, 64:128])
            w3T = (wts[0:64, 128:192], wts[64:128, 128:192])
            w4bd = cons.tile([128, 2], f16, tag="w4bd")

            c1 = cons.tile([1, C1W], f32, tag="c1")
            w4row = c1[:, 384:386]
            b4row = c1[:, 386:388]
            ones2row = c1[:, 388:390]

            LA = cons.tile([N, 3 * H], f16, tag="LA")
            LB2 = cons.tile([N, 6 * H], f16, tag="LB2")  # r|r|z|z|n|n
            ones_col = cons.tile([128, 1], f32, tag="ones_col")
            onecell = ones_col[0:1, 0:1]
            b1col = cons.tile([128, 1], f32, tag="b1col")
            c2col = cons.tile([128, 1], f32, tag="c2col")
            c3col = cons.tile([128, 1], f32, tag="c3col")

            scmb_t = []
            for ci, (c0, cw) in enumerate(CHUNKS):
                ns = 3 if ci < NSH else 4
                st = spool.tile([N, ns * cw], f16, tag=f"scmb{ci}",
                                name=f"scmb{ci}")
                scmb_t.append(st)

            y1T = big.tile([128, F_PAD], f16, tag="y1T")
            y2T = big.tile([128, F_PAD], f16, tag="y2T")
            y3T = big.tile([128, F_PAD], f16, tag="y3T")
            oT = big.tile([2, F], f32, tag="oT")
            ST1 = big.tile([128, 4], f32, tag="ST1")
            ST2 = big.tile([128, 4], f32, tag="ST2")
            ST3 = big.tile([128, 8], f32, tag="ST3")

            # ---- input DMAs: critical-first, 2 HWDGE queues + SWDGE ----
            nc.sync.dma_start(pk[0:42, :], pkA_d.ap()[0:42, :])
            nc.sync.dma_start(scmb_t[0][0:42, :], scmb_d[0].ap()[0:42, :])
            nc.sync.dma_start(scmb_t[2][0:42, :], scmb_d[2].ap()[0:42, :])
            nc.scalar.dma_start(pk[42:N, :], pkA_d.ap()[42:N, :])
            nc.scalar.dma_start(scmb_t[0][42:N, :], scmb_d[0].ap()[42:N, :])
            nc.scalar.dma_start(scmb_t[2][42:N, :], scmb_d[2].ap()[42:N, :])
            nc.gpsimd.dma_start(c1[:], c1_d.ap())
            nc.gpsimd.dma_start(scmb_t[1][:], scmb_d[1].ap())
            nc.gpsimd.dma_start(wts[0:64, :], wts_d.ap())
            nc.gpsimd.dma_start(wts[64:128, :], wts_d.ap())
            nc.gpsimd.dma_start(scmb_t[3][:], scmb_d[3].ap())

            # table preload: dummy sigmoid on a memset cell (no DMA dep)
            wsrc = nrp.tile([1, 1], f32, tag="wsrc")
            nc.vector.memset(wsrc[:], 0.0)
            warm = nrp.tile([1, 1], f32, tag="warm")
            nc.scalar.activation(warm[:], wsrc[:], AF.Sigmoid)

            nc.vector.memset(ones_col[:], 1.0)

            # ---- A = [x|1]@[W_ih.T;b_ih], B likewise (PE-first: critical) --
            pA0 = psm.tile([N, 3 * H], f32, tag="p_l", padded_shape=[N, 512],
                           name="pA0")
            nc.tensor.matmul(pA0[:], xT_aug, wih_aug, start=True, stop=True)
            nc.vector.tensor_scalar(LA[:], pA0[:], 1.0, None, OP.mult)
            pB0 = psm.tile([N, 3 * H], f32, tag="p_l", padded_shape=[N, 512],
                           name="pB0")
            nc.tensor.matmul(pB0[:], xT_aug, whh_aug, start=True, stop=True)
            for g in range(3):
                gs = slice(64 * g, 64 * g + 64)
                nc.vector.tensor_scalar(LB2[:, 128 * g:128 * g + 64],
                                        pB0[:, gs], 1.0, None, OP.mult)
                nc.vector.tensor_scalar(LB2[:, 128 * g + 64:128 * g + 128],
                                        pB0[:, gs], 1.0, None, OP.mult)

            # b1col transpose (needed by first L1 evacuation)
            p_b1 = psm.tile([128, 1], f32, tag="p_l", padded_shape=[128, 512],
                            name="p_b1")
            nc.tensor.matmul(p_b1[:], c1[:, 0:128], onecell, start=True,
                             stop=True)
            nc.vector.tensor_scalar(b1col[:], p_b1[:], 1.0, None, OP.mult)

            # ---- GRU + L1, chunk by chunk (emission software-pipelined) ----
            PO = (slice(0, 64), slice(64, 128))
            TP = ((0, 0), (0, 64))

            def gru_chunk_mm(ci):
                c0, cw = CHUNKS[ci]
                st = scmb_t[ci]
                p_rz = psrz.tile([128, 1024], f32, tag="p_rz")
                p_An = psnb.tile([128, cw], f32, tag="p_An",
                                 padded_shape=[128, 512])
                p_Bn = psnb.tile([128, cw], f32, tag="p_Bn",
                                 padded_shape=[128, 512])
                p_x2 = psnb.tile([128, cw], f32, tag="p_x2",
                                 padded_shape=[128, 512])

                if ci < NSH:
                    siu = (st[:, 0:cw], st[:, cw:2 * cw])
                    sju = st[:, 2 * cw:3 * cw]
                    for g, Lsl in ((0, slice(0, 64)), (1, slice(64, 128))):
                        for hi in range(2):
                            nc.tensor.matmul(p_rz[PO[hi], 512 * g:512 * g + cw],
                                             LA[:, Lsl], siu[hi],
                                             start=True, stop=False,
                                             tile_position=TP[hi],
                                             skip_group_check=True)
                        nc.tensor.matmul(p_rz[:, 512 * g:512 * g + cw],
                                         LB2[:, 128 * g:128 * g + 128], sju,
                                         start=False, stop=True,
                                         skip_group_check=True)
                    for hi in range(2):
                        nc.tensor.matmul(p_An[PO[hi], :], LA[:, 128:192],
                                         siu[hi], start=True, stop=True,
                                         tile_position=TP[hi],
                                         skip_group_check=True)
                    nc.tensor.matmul(p_Bn[:], LB2[:, 256:384], sju,
                                     start=True, stop=True,
                                     skip_group_check=True)
                    nc.tensor.matmul(p_x2[:], x2dup, sju, start=True,
                                     stop=True, skip_group_check=True)
                else:
                    siu = (st[:, 0:cw], st[:, 2 * cw:3 * cw])
                    sju = (st[:, cw:2 * cw], st[:, 3 * cw:4 * cw])
                    for g, Lsl in ((0, slice(0, 64)), (1, slice(64, 128))):
                        for hi in range(2):
                            nc.tensor.matmul(p_rz[PO[hi], 512 * g:512 * g + cw],
                                             LA[:, Lsl], siu[hi],
                                             start=True, stop=False,
                                             tile_position=TP[hi],
                                             skip_group_check=True)
                            nc.tensor.matmul(p_rz[PO[hi], 512 * g:512 * g + cw],
                                             LB2[:, 128 * g:128 * g + 64],
                                             sju[hi],
                                             start=False, stop=True,
                                             tile_position=TP[hi],
                                             skip_group_check=True)
                    for hi in range(2):
                        nc.tensor.matmul(p_An[PO[hi], :], LA[:, 128:192],
                                         siu[hi], start=True, stop=True,
                                         tile_position=TP[hi],
                                         skip_group_check=True)
                        nc.tensor.matmul(p_Bn[PO[hi], :], LB2[:, 256:320],
                                         sju[hi], start=True, stop=True,
                                         tile_position=TP[hi],
                                         skip_group_check=True)
                        nc.tensor.matmul(p_x2[PO[hi], :], x_t, sju[hi],
                                         start=True, stop=True,
                                         tile_position=TP[hi],
                                         skip_group_check=True)
                return p_rz, p_An, p_Bn, p_x2

            def gru_chunk_ew(ci, p_rz, p_An, p_Bn, p_x2):
                c0, cw = CHUNKS[ci]
                csl = slice(c0, c0 + cw)
                rz_c = scr.tile([128, 2 * cw], f16, tag="rz", name="rz")
                s_c = scr.tile([128, cw], f16, tag="s")
                s2_c = scr.tile([128, cw], f16, tag="s2")
                nn_c = scr.tile([128, cw], f16, tag="nn")
                zx2_c = scr.tile([128, cw], f16, tag="zx2")
                q_c = scr.tile([128, cw], f16, tag="q")
                h_c = scr.tile([128, cw], f16, tag="h")

                rz_src = p_rz[:].rearrange("p (b k) -> p b k", b=2)[:, :, 0:cw]
                rz_dst = rz_c[:].rearrange("p (b k) -> p b k", b=2)
                r_sl = rz_c[:, 0:cw]
                z_sl = rz_c[:, cw:2 * cw]

                nc.scalar.activation(rz_dst, rz_src, AF.Sigmoid)
                nc.vector.tensor_tensor(s_c[:], r_sl, p_Bn[:], OP.mult)
                nc.vector.tensor_tensor(s2_c[:], s_c[:], p_An[:], OP.add)
                nc.scalar.activation(nn_c[:], s2_c[:], AF.Tanh)
                nc.vector.tensor_tensor(zx2_c[:], z_sl, p_x2[:], OP.mult)
                nc.vector.scalar_tensor_tensor(q_c[:], z_sl, 1.0, nn_c[:],
                                               OP.subtract, OP.mult)
                nc.gpsimd.tensor_tensor(h_c[:], zx2_c[:], q_c[:], OP.subtract)

                p_l1 = psm.tile([128, cw], f32, tag="p_l",
                                padded_shape=[128, 512], name=f"p_l1{ci}")
                for hi in range(2):
                    nc.tensor.matmul(p_l1[PO[hi], :], w1T[hi], h_c[PO[hi], :],
                                     start=True, stop=True,
                                     tile_position=(64 * hi, 64 * hi),
                                     skip_group_check=True)
                nc.scalar.activation(y1T[:, csl], p_l1[:], AF.Relu,
                                     bias=b1col[:],
                                     accum_out=ST1[:, ci:ci + 1])
                if dbg:
                    dbg_h = scr.tile([128, cw], f32, tag="dbgh", name="dbg_h")
                    nc.vector.tensor_scalar(dbg_h[:], h_c[:], 1.0, None,
                                            OP.mult)
                    nc.sync.dma_start(dbg_d["h"].ap()[:, csl], dbg_h[:])

            pending = None
            for ci in range(len(CHUNKS)):
                ps = gru_chunk_mm(ci)
                if pending is not None:
                    gru_chunk_ew(pending[0], *pending[1])
                pending = (ci, ps)
            gru_chunk_ew(pending[0], *pending[1])

            def lpool(ci, shape, nm):
                """Alternate L-layer PSUM between psm:p_l and psnb:p_x2."""
                if ci % 2 == 0:
                    return psm.tile(shape, f32, tag="p_l",
                                    padded_shape=[shape[0], 512], name=nm)
                return psnb.tile(shape, f32, tag="p_x2",
                                 padded_shape=[128, 512], name=nm)

            # ---- chain1: m1 only -> c2col = -m1*rowsum(W2) ----
            p_s1 = psnb.tile([1, 4], f32, tag="p_x2",
                             padded_shape=[128, 512], name="p_s1")
            nc.tensor.matmul(p_s1[:], ones_col[:], ST1[:], start=True,
                             stop=True)
            s1 = nrp.tile([1, 1], f32, tag="s1")
            nc.vector.tensor_reduce(s1[:], p_s1[:], axis=mybir.AxisListType.X,
                                    op=OP.add)
            m1n = nrp.tile([1, 1], f32, tag="m1n")
            nc.vector.tensor_scalar(m1n[:], s1[:], -1.0 / (M * 64.0), None,
                                    OP.mult)
            p_c2 = psnb.tile([128, 1], f32, tag="p_An",
                             padded_shape=[128, 512], name="p_c2")
            nc.tensor.matmul(p_c2[:], c1[:, 128:256], m1n[:], start=True,
                             stop=True)
            nc.vector.tensor_scalar(c2col[:], p_c2[:], 1.0, None, OP.mult)

            # ---- L2 ----
            for ci, (c0, cw) in enumerate(CHUNKS):
                csl = slice(c0, c0 + cw)
                p_l2 = lpool(ci, [128, cw], f"p_l2{ci}")
                for hi in range(2):
                    nc.tensor.matmul(p_l2[PO[hi], :], w2T[hi],
                                     y1T[PO[hi], csl], start=True, stop=True,
                                     tile_position=(64 * hi, 64 * hi),
                                     skip_group_check=True)
                nc.scalar.activation(y2T[:, csl], p_l2[:], AF.Relu,
                                     bias=c2col[:],
                                     accum_out=ST2[:, ci:ci + 1])

            # ---- chain2: m^2 only -> c3col ----
            p_s2 = psnb.tile([1, 4], f32, tag="p_Bn",
                             padded_shape=[128, 512], name="p_s2")
            nc.tensor.matmul(p_s2[:], ones_col[:], ST2[:], start=True,
                             stop=True)
            s2s = nrp.tile([1, 1], f32, tag="s2s")
            nc.vector.tensor_reduce(s2s[:], p_s2[:], axis=mybir.AxisListType.X,
                                    op=OP.add)
            m2n = nrp.tile([1, 1], f32, tag="m2n")
            nc.vector.tensor_scalar(m2n[:], s2s[:], -1.0 / (M * 32.0), None,
                                    OP.mult)
            p_c3 = psnb.tile([128, 1], f32, tag="p_An",
                             padded_shape=[128, 512], name="p_c3")
            nc.tensor.matmul(p_c3[:], c1[:, 256:384], m2n[:], start=True,
                             stop=True)
            nc.vector.tensor_scalar(c3col[:], p_c3[:], 1.0, None, OP.mult)

            # ---- L3 (+ sum-of-squares pass on DVE for LN3 stats) ----
            for ci, (c0, cw) in enumerate(CHUNKS):
                csl = slice(c0, c0 + cw)
                p_l3 = lpool(ci, [128, cw], f"p_l3{ci}")
                for hi in range(2):
                    nc.tensor.matmul(p_l3[PO[hi], :], w3T[hi],
                                     y2T[PO[hi], csl], start=True, stop=True,
                                     tile_position=(64 * hi, 64 * hi),
                                     skip_group_check=True)
                nc.scalar.activation(y3T[:, csl], p_l3[:], AF.Relu,
                                     bias=c3col[:],
                                     accum_out=ST3[:, ci:ci + 1])
                dump = scr.tile([128, cw], f16, tag="dump", name="dump")
                nc.vector.scalar_tensor_tensor(dump[:], y3T[:, csl], 0.0,
                                               y3T[:, csl], OP.add, OP.mult,
                                               accum_out=ST3[:, 4 + ci:5 + ci])

            # w4 block-diag columns (off critical path, before L4)
            p_w4a = psm.tile([128, 1], f32, tag="p_l", padded_shape=[128, 512],
                             name="p_w4a")
            nc.tensor.matmul(p_w4a[:], c1[:, 390:518], onecell, start=True,
                             stop=True)
            nc.vector.tensor_scalar(w4bd[:, 0:1], p_w4a[:], 1.0, None, OP.mult)
            p_w4b = psm.tile([128, 1], f32, tag="p_l", padded_shape=[128, 512],
                             name="p_w4b")
            nc.tensor.matmul(p_w4b[:], c1[:, 518:646], onecell, start=True,
                             stop=True)
            nc.vector.tensor_scalar(w4bd[:, 1:2], p_w4b[:], 1.0, None, OP.mult)

            # ---- chain3: mean+var -> a3 (single rsqrt via recip+Newton) ----
            p_s3 = psnb.tile([1, 8], f32, tag="p_Bn",
                             padded_shape=[128, 512], name="p_s3")
            nc.tensor.matmul(p_s3[:], ones_col[:], ST3[:], start=True,
                             stop=True)

            # L4 matmuls for chunks 0/1 can run during the scalar chain
            p_l4 = {}
            for ci in (0, 1):
                c0, cw = CHUNKS[ci]
                p_l4[ci] = lpool(ci, [2, cw], f"p_l4{ci}")
                nc.tensor.matmul(p_l4[ci][:], w4bd[:],
                                 y3T[:, c0:c0 + cw], start=True, stop=True)

            sums = nrp.tile([1, 2], f32, tag="sums")
            nc.vector.tensor_reduce(
                sums[:], p_s3[:].rearrange("p (a b) -> p a b", a=2),
                axis=mybir.AxisListType.X, op=OP.add)
            mq = nrp.tile([1, 2], f32, tag="mq")
            nc.vector.tensor_scalar(mq[:], sums[:], 1.0 / (M * 32.0), None,
                                    OP.mult)
            m2 = nrp.tile([1, 1], f32, tag="m2")
            nc.vector.tensor_scalar(m2[:], mq[:, 0:1], mq[:, 0:1], None,
                                    OP.mult)
            v_t = nrp.tile([1, 1], f32, tag="v")
            nc.vector.scalar_tensor_tensor(v_t[:], m2[:], -1.0, mq[:, 1:2],
                                           OP.mult, OP.add)
            nc.vector.tensor_scalar(v_t[:], v_t[:], 1.0, EPS, OP.mult, OP.add)
            vqs = []
            for k in range(NR_ITERS):
                vq = nrp.tile([1, 1], f32, tag=f"vq{k}", name=f"vq{k}")
                nc.vector.tensor_scalar(vq[:], v_t[:], 0.25 ** k, None,
                                        OP.mult)
                vqs.append(vq)
            rv = nrp.tile([1, 1], f32, tag="rv")
            nc.vector.reciprocal(rv[:], v_t[:])
            t1 = nrp.tile([1, 1], f32, tag="t1")
            nc.vector.tensor_scalar(t1[:], v_t[:], RC, RB, OP.mult, OP.add)
            w_t = nrp.tile([1, 1], f32, tag="w")
            nc.vector.scalar_tensor_tensor(w_t[:], rv[:], RA, t1[:],
                                           OP.mult, OP.add)
            t_t = nrp.tile([1, 1], f32, tag="t")
            for k in range(NR_ITERS):
                nc.vector.tensor_scalar(t_t[:], w_t[:], w_t[:], vqs[k][:],
                                        OP.mult, OP.mult)
                nc.vector.scalar_tensor_tensor(w_t[:], t_t[:], 3.0, w_t[:],
                                               OP.subtract, OP.mult)
            G = nrp.tile([1, 1], f32, tag="G")
            nc.vector.tensor_scalar(G[:], w_t[:], (-0.5) ** NR_ITERS, None,
                                    OP.mult)
            # scale4 = a3 on both partitions; bias4 = -a3*m3*rowsum(W4) + b4
            A4 = nrp.tile([1, 1], f32, tag="A4")
            nc.vector.tensor_scalar(A4[:], mq[:, 0:1], G[:], -1.0,
                                    OP.mult, OP.mult)
            p_s4 = psnb.tile([2, 2], f32, tag="p_An",
                             padded_shape=[128, 512], name="p_s4")
            nc.tensor.matmul(p_s4[:, 0:1], ones2row, G[:], start=True,
                             stop=True)
            nc.tensor.matmul(p_s4[:, 1:2], w4row, A4[:], start=True,
                             stop=False)
            nc.tensor.matmul(p_s4[:, 1:2], b4row, onecell, start=False,
                             stop=True)
            sc4 = nrp.tile([2, 2], f32, tag="sc4")
            nc.vector.tensor_scalar(sc4[:], p_s4[:], 1.0, None, OP.mult)
            scale4 = sc4[:, 0:1]
            bias4 = sc4[:, 1:2]

            # ---- L4 + sigmoid + output ----
            for ci, (c0, cw) in enumerate(CHUNKS):
                csl = slice(c0, c0 + cw)
                if ci not in p_l4:
                    p_l4[ci] = lpool(ci, [2, cw], f"p_l4{ci}")
                    nc.tensor.matmul(p_l4[ci][:], w4bd[:], y3T[:, csl],
                                     start=True, stop=True)
                nc.scalar.activation(oT[:, csl], p_l4[ci][:], AF.Sigmoid,
                                     bias=bias4, scale=scale4)
                nc.sync.dma_start(out_d.ap()[:, csl], oT[:, csl])
            if dbg:
                for nm, t in [("y1", y1T), ("y2", y2T), ("y3", y3T)]:
                    dt_ = big.tile([128, F], f32, tag="dbgy" + nm,
                                   name="dbgy" + nm)
                    nc.vector.tensor_scalar(dt_[:], t[0:128, 0:F], 1.0, None,
                                            OP.mult)
                    nc.sync.dma_start(dbg_d[nm].ap(), dt_[:])
                nc.sync.dma_start(dbg_d["ST1"].ap(), ST1[:])
                nc.sync.dma_start(dbg_d["ST2"].ap(), ST2[:])
                nc.sync.dma_start(dbg_d["ST3"].ap(), ST3[:])

    nc.compile()
    return nc


def _host_inputs(inputs):
    """Build the device input map from the raw model inputs."""
    x = np.ascontiguousarray(inputs["x"], np.float32)
    W_ih = np.asarray(inputs["W_ih"], np.float32)
    W_hh = np.asarray(inputs["W_hh"], np.float32)
    b_ih = np.asarray(inputs["b_ih"], np.float32)
    b_hh = np.asarray(inputs["b_hh"], np.float32)
    W1 = np.asarray(inputs["W1"], np.float32)
    b1 = np.asarray(inputs["b1"], np.float32)
    W2 = np.asarray(inputs["W2"], np.float32)
    W3 = np.asarray(inputs["W3"], np.float32)
    W4 = np.asarray(inputs["W4"], np.float32)
    b4 = np.asarray(inputs["b4"], np.float32)
    f16 = np.float16

    pk = np.zeros((N, PKW), f16)
    pk[0:H, 0:84] = x.T
    pk[H, 0:84] = 1.0
    pk[0:N, 84:148] = x
    pk[0:N, 148:212] = x
    pk[0:H, 212:404] = W_ih.T
    pk[H, 212:404] = b_ih
    pk[0:H, 404:596] = W_hh.T
    pk[H, 404:596] = b_hh

    wt = np.zeros((H, WTW), f16)
    wt[0:64, 0:64] = W1.T
    wt[0:64, 64:96] = W2.T          # [64, 32]; cols 96:128 stay 0
    wt[0:32, 128:160] = W3.T        # [32, 32]; rest 0

    c1 = np.zeros((1, C1W), np.float32)
    c1[0, 0:64] = b1
    c1[0, 64:128] = b1
    c1[0, 128:160] = W2.sum(1)
    c1[0, 192:224] = W2.sum(1)
    c1[0, 256:288] = W3.sum(1)
    c1[0, 320:352] = W3.sum(1)
    c1[0, 384:386] = W4.sum(1)
    c1[0, 386:388] = b4
    c1[0, 388:390] = 1.0
    c1[0, 390:422] = W4[0, :]       # w4bd col 0, partitions 0:32
    c1[0, 582:614] = W4[0, :]       # w4bd col 1, partitions 64:96

    def onehot(idx):
        S = np.zeros((N, F), f16)
        S[idx, np.arange(F)] = 1.0
        return S

    siu0, sju0 = onehot(_IU0), onehot(_JU0)
    siu1, sju1 = onehot(_IU1), onehot(_JU1)

    out = {"pkA": pk, "wtsA": wt, "consts1": c1}
    for ci, (c0, cw) in enumerate(CHUNKS):
        sl = slice(c0, c0 + cw)
        if ci < NSH:
            sc = np.concatenate([siu0[:, sl], siu1[:, sl], sju0[:, sl]],
                                axis=1)
        else:
            sc = np.concatenate([siu0[:, sl], sju0[:, sl],
                                 siu1[:, sl], sju1[:, sl]], axis=1)
        if ci in (1, 3):
            out[f"scmb{ci}"] = np.ascontiguousarray(sc).astype(np.uint8)
        else:
            out[f"scmb{ci}"] = np.ascontiguousarray(sc)
    return out


def _assemble(o_packed):
    A = np.zeros((N, N), np.float32)
    A[_IU0, _JU0] = o_packed[0]
    A[_IU1, _JU1] = o_packed[1]
    return A + A.T


def _supported(inputs):
    """The fast path folds away identity LayerNorm affines and requires
    b2 == b3 == 0 (true for the canonical setup_inputs)."""
    for g in ("g1", "g2", "g3"):
        if g in inputs and not np.all(np.asarray(inputs[g]) == 1.0):
            return False
    for b in ("be1", "be2", "be3", "b2", "b3"):
        if b in inputs and not np.all(np.asarray(inputs[b]) == 0.0):
            return False
    return True


def _numpy_reference(inputs):
    """Generic fallback (non-identity LayerNorm affine params only)."""
    x = np.asarray(inputs["x"], np.float64)
    iu, ju = np.triu_indices(N, k=1)
    gi = x[iu] @ np.asarray(inputs["W_ih"]).T + np.asarray(inputs["b_ih"])
    gh = x[ju] @ np.asarray(inputs["W_hh"]).T + np.asarray(inputs["b_hh"])
    i_r, i_z, i_n = np.split(gi, 3, 1)
    h_r, h_z, h_n = np.split(gh, 3, 1)
    r = 1 / (1 + np.exp(-(i_r + h_r)))
    z = 1 / (1 + np.exp(-(i_z + h_z)))
    nn_ = np.tanh(i_n + r * h_n)
    h = (1 - z) * nn_ + z * x[ju]

    def ln(y, g, b):
        m = y.mean()
        v = ((y - m) ** 2).mean()
        return (y - m) / np.sqrt(v + EPS) * np.asarray(g) + np.asarray(b)

    h = ln(np.maximum(h @ np.asarray(inputs["W1"]).T + np.asarray(inputs["b1"]), 0),
           inputs["g1"], inputs["be1"])
    h = ln(np.maximum(h @ np.asarray(inputs["W2"]).T + np.asarray(inputs["b2"]), 0),
           inputs["g2"], inputs["be2"])
    h = ln(np.maximum(h @ np.asarray(inputs["W3"]).T + np.asarray(inputs["b3"]), 0),
           inputs["g3"], inputs["be3"])
    o = 1 / (1 + np.exp(-(h @ np.asarray(inputs["W4"]).T + np.asarray(inputs["b4"]))))
    A = np.zeros((N, N), np.float32)
    A[iu, ju] = o[:, 0]
    return A + A.T


def kernel(**inputs):
    if not _supported(inputs):
        return _numpy_reference(inputs)

    if "nc" not in _prog_cache:
        _prog_cache["nc"] = _build_program()
    nc = _prog_cache["nc"]

    from concourse.bass_utils import run_bass_kernel_spmd

    in_map = _host_inputs(inputs)
    res = run_bass_kernel_spmd(nc, [in_map], core_ids=[0])
    return _assemble(res.results[0]["o"])


if __name__ == "__main__":
    sys.path.insert(0, os.path.dirname(os.path.abspath(__file__)))
    import jax
    jax.config.update("jax_platforms", "cpu")
    import reference

    ins = {k: np.asarray(v) for k, v in reference.setup_inputs().items()}
    expected = np.asarray(reference.reference(**ins))
    got = kernel(**ins)
    err = np.abs(got - expected).max()
    print("absmax err:", err, "rel:", err / np.abs(expected).max())
